# revision 16
# baseline (speedup 1.0000x reference)
"""Trainium2 kernel for ChannelQuadLayer.

Per-pixel quadratic channel expansion + 1x1 conv:
    quad = x[:, ii] * x[:, jj]  (all 2080 upper-tri channel pairs)
    y    = concat([x, quad])    -> [B, 2144, H, W]
    out  = einsum('bchw,oc->bohw', y, fc_w)

Strategy (8 NeuronCores, batch-parallel, one sample per core):
  * The 2080 unordered channel pairs are exactly the cyclic diagonals
    d=0..32 of the 64-channel index ring: pairs {i, (i+d)%64}.
  * Host prepares 9 "rotation buffers" B_k = [roll(x,-t_k); roll(x,-u_k)]
    (128 partitions x 4096 pixels, bf16). A single elementwise multiply
    of two such buffers yields TWO complete cyclic diagonals (top half:
    diagonal t_j - t_i, bottom half: u_j - u_i). A difference cover
    (found by search) produces all diagonals 1..32 in 16 multiplies;
    diagonal 0 (squares) comes from one ScalarE Square op.
  * Everything on the input path is bf16: halves HBM traffic (the 9
    rotation buffers are the dominant stream) and doubles VectorE
    multiply throughput; the matmul accumulates in fp32 PSUM so the
    output error stays ~1e-3 (tolerance 2e-2).
  * The 9 buffers are packed pass-major in ONE dram tensor so each
    pixel pass needs only 3 DMA triggers (b0 | b1-4 | b5-8) instead
    of 9 - HWDGE trigger instructions cost ~650ns each on the queue.
    All input DMAs are issued from nc.sync: one HWDGE ring = FIFO, so
    critical early transfers are not bandwidth-shared with prefetches
    (different rings round-robin per packet). Outputs go on the other
    ring (nc.scalar).
  * y-rows: 64 linear + 64 squares + 16*128 pair rows = 2176 = 17*128,
    an exact 17-chunk contraction. fc_w is permuted/padded to this row
    order on the host (duplicate pair rows get zero weight).
  * GEMM: out[256, 4096] = Wt[2176, 256]^T @ y[2176, 4096] on TensorE
    in bf16, accumulating 17 chunks into fp32 PSUM, k-outer so each y
    chunk is consumed right after its producer. Weights DMA in 4
    pieces (chunk 0 | 1-4 | 5-8 | 9-16) interleaved with the pass-0
    buffer pieces in descending urgency, so the first matmul only
    waits on a 64KB transfer.
  * The m=1 PSUM drains run on VectorE so the scalar queue stays free
    for the next pass's chunk-0 activations at pass boundaries.
  * 5 warm-up matmuls on a memset tile bridge the DMA ramp so the PE
    HAM clock-gate (default 1.2 GHz) releases to 2.4 GHz with no idle
    window before the real accumulation starts.
  * Pixel passes [256, 512, 1024, 1024, 1024, 256]: small first passes
    minimize the bytes the PE waits on at startup, the small last pass
    shortens the drain/writeback tail.
"""

import sys

sys.path.insert(0, "/opt/trn_rl_repo")

import ml_dtypes
import numpy as np

import concourse.bass as bass
import concourse.tile as tile
from concourse import bacc, mybir
from concourse.bass_utils import run_bass_kernel_spmd

B, C, H, W = 8, 64, 64, 64
PIX = H * W  # 4096
OUT = 256
NCORES = 8

# rotation difference cover: ops (i,j) give diagonals D(t_j-t_i) (top half)
# and D(u_j-u_i) (bottom half); together exactly {1..32}.
T_ROT = [0, 8, 22, 24, 42, 48, 49, 57, 60]
U_ROT = [0, 59, 16, 38, 55, 22, 30, 54, 35]
OPS = [(1, 3), (2, 3), (1, 4), (2, 4), (3, 4), (4, 5), (1, 6), (2, 6),
       (6, 7), (0, 7), (4, 7), (5, 7), (2, 8), (3, 8), (5, 8), (6, 8)]
NB = len(T_ROT)        # 9 rotation buffers
KCH = 1 + len(OPS)     # 17 contraction chunks of 128 rows
PASS_FD = [256, 512, 1024, 1024, 1024, 256]
assert sum(PASS_FD) == PIX
NPASS = len(PASS_FD)

F32 = mybir.dt.float32
BF16 = mybir.dt.bfloat16
BF16_NP = ml_dtypes.bfloat16

# b-buffer groups: one DMA per group per pass (contiguous in the packed
# dram layout). b0 alone so chunk0 can start as early as possible.
BGRP = [(0, 1), (1, 5), (5, 9)]


def row_pairs():
    """Channel pair (c1, c2) for every global y row, or ('lin', c)."""
    rows = []
    for p in range(128):  # chunk 0
        rows.append(("lin", p) if p < 64 else (p - 64, p - 64))
    for (i, j) in OPS:
        for p in range(128):
            if p < 64:
                c1, c2 = (p + T_ROT[i]) % 64, (p + T_ROT[j]) % 64
            else:
                c1, c2 = (p - 64 + U_ROT[i]) % 64, (p - 64 + U_ROT[j]) % 64
            rows.append((min(c1, c2), max(c1, c2)))
    return rows


def build_wt(fc_w):
    """Permute fc_w [OUT, 2144] into Wt [KCH, 128, OUT] matching y rows."""
    ii, jj = np.triu_indices(C)
    pair2col = {(a, b): C + k for k, (a, b) in enumerate(zip(ii, jj))}
    wt = np.zeros((KCH * 128, OUT), np.float32)
    seen = set()
    for g, r in enumerate(row_pairs()):
        if r[0] == "lin":
            wt[g] = fc_w[:, r[1]]
        elif r not in seen:
            seen.add(r)
            wt[g] = fc_w[:, pair2col[r]]
    assert len(seen) == C * (C + 1) // 2
    return np.ascontiguousarray(wt.reshape(KCH, 128, OUT))


_cached = None


def _build_module():
    global _cached
    if _cached is not None:
        return _cached
    nc = bacc.Bacc("TRN2", target_bir_lowering=False, debug=False,
                   num_devices=NCORES)
    # rotation buffers, pass-major packed: per pass p (pixel range
    # [off, off+FD)), columns [9*off + j*FD, 9*off + (j+1)*FD) hold
    # buffer j's slice.
    bp_d = nc.dram_tensor("bpack", [128, NB * PIX], BF16, kind="ExternalInput")
    # weight matrix, partition-major so DMA rows are contiguous
    wt_d = nc.dram_tensor("wt", [128, KCH * OUT], BF16, kind="ExternalInput")
    out_d = nc.dram_tensor("out", [2, 128, PIX], BF16, kind="ExternalOutput")

    # wt DMA split, in chunks; piece 0 (chunk 0) is tiny so the first
    # real matmul only waits on a 64KB transfer. All INPUT DMAs go on
    # the nc.sync HWDGE ring: one ring = FIFO, so earlier (critical)
    # transfers are not bandwidth-shared with later prefetches.
    WPIECE = [(0, 1), (1, 5), (5, 9), (9, KCH)]

    with tile.TileContext(nc) as tc:
        with tc.tile_pool(name="wt", bufs=1) as wt_pool, \
             tc.tile_pool(name="warm", bufs=1) as warm_pool, \
             tc.tile_pool(name="bsrc", bufs=2) as b_pool, \
             tc.tile_pool(name="y", bufs=8) as y_pool, \
             tc.tile_pool(name="ostage", bufs=4) as o_pool, \
             tc.tile_pool(name="psum", bufs=8, space="PSUM") as ps_pool:

            wt_t = wt_pool.tile([128, KCH * OUT], BF16, name="wtt")

            # PE warm-up: ~5 cold matmuls bridge the DMA ramp so the HAM
            # clock gate sees activity early. Results are never read.
            wz = warm_pool.tile([128, 512], BF16, name="warmz")
            nc.gpsimd.memset(wz[:, :], 0.0)
            ps_w = ps_pool.tile([128, 512], F32, tag="ps", name="ps_warm")
            for _ in range(5):
                nc.tensor.matmul(ps_w[:, :], wz[:, 0:128], wz[:, :],
                                 start=True, stop=True)

            def wt_dma(piece):
                k0, k1 = WPIECE[piece]
                nc.sync.dma_start(wt_t[:, k0 * OUT:k1 * OUT],
                                  wt_d.ap()[:, k0 * OUT:k1 * OUT])

            wt_dma(0)

            off = 0
            for ps, FD in enumerate(PASS_FD):
                NT = max(1, FD // 512)
                NW = min(512, FD)  # matmul free width
                base = NB * off
                # one tile per buffer-group; 3 DMA triggers per pass,
                # FIFO-ordered so the critical pieces land first
                gt = []
                for gi, (j0, j1) in enumerate(BGRP):
                    t = b_pool.tile([128, (j1 - j0) * 1024], BF16,
                                    tag=f"bg{gi}", name=f"bg{gi}_{ps}")
                    nc.sync.dma_start(
                        t[:, :(j1 - j0) * FD],
                        bp_d.ap()[:, base + j0 * FD:base + j1 * FD])
                    gt.append(t)
                    if ps == 0 and gi == 1:
                        wt_dma(1)
                        wt_dma(2)
                    if ps == 0 and gi == 2:
                        wt_dma(3)

                def bview(j):
                    for gi, (j0, j1) in enumerate(BGRP):
                        if j0 <= j < j1:
                            return gt[gi][:, (j - j0) * FD:(j - j0 + 1) * FD]
                    raise AssertionError

                psum = [ps_pool.tile([128, 512], F32, tag="ps",
                                     name=f"ps{ps}_{g}")
                        for g in range(2 * NT)]

                for k in range(KCH):
                    yk = y_pool.tile([128, 1024], BF16, tag="y",
                                     name=f"y{ps}_{k}")
                    if k == 0:
                        # linear rows + squares, both from the b0 tile.
                        # In pass 0 the copy goes on VectorE so the two
                        # halves build concurrently (startup critical
                        # path); later passes keep both on ScalarE so
                        # the VectorE multiply pipeline is not perturbed
                        # at pass boundaries.
                        b0 = gt[0]
                        if ps == 0:
                            nc.vector.tensor_copy(yk[0:64, :FD], b0[0:64, :FD])
                        else:
                            nc.scalar.activation(
                                yk[0:64, :FD], b0[0:64, :FD],
                                mybir.ActivationFunctionType.Identity)
                        nc.scalar.activation(
                            yk[64:128, :FD], b0[64:128, :FD],
                            mybir.ActivationFunctionType.Square)
                    else:
                        i, j = OPS[k - 1]
                        nc.vector.tensor_mul(yk[:, :FD], bview(i), bview(j))
                    for m in range(2):
                        lhsT = wt_t[:, k * OUT + m * 128:k * OUT + (m + 1) * 128]
                        for n in range(NT):
                            nc.tensor.matmul(
                                psum[m * NT + n][:, :NW],
                                lhsT,
                                yk[:, n * NW:(n + 1) * NW],
                                start=(k == 0), stop=(k == KCH - 1))

                last = ps == NPASS - 1
                for m in range(2):
                    ot = o_pool.tile([128, 1024], BF16, tag="ostage",
                                     name=f"o{ps}_{m}")
                    for n in range(NT):
                        src = psum[m * NT + n][:, :NW]
                        dst = ot[:, n * NW:(n + 1) * NW]
                        # m1 drains on VectorE so the scalar queue is free
                        # for the next pass's chunk-0 activations at the
                        # pass boundary; m0 stays on ScalarE.
                        if m == 1:
                            nc.vector.tensor_copy(dst, src)
                        else:
                            nc.scalar.activation(
                                dst, src, mybir.ActivationFunctionType.Identity)
                    eng = nc.sync if (last and m == 1) else nc.scalar
                    eng.dma_start(out_d.ap()[m, :, off:off + FD], ot[:, :FD])
                off += FD
    nc.compile()
    _cached = nc
    return nc


def make_in_maps(x, wt):
    # [KCH, 128, OUT] -> [128, KCH*OUT], bf16
    wtp = np.ascontiguousarray(
        wt.transpose(1, 0, 2).reshape(128, KCH * OUT).astype(BF16_NP))
    in_maps = []
    for b in range(B):
        xc = np.asarray(x[b], np.float32).reshape(C, PIX).astype(BF16_NP)
        bufs = [np.concatenate(
            [np.roll(xc, -t, axis=0), np.roll(xc, -u, axis=0)])
            for t, u in zip(T_ROT, U_ROT)]
        # pass-major packing: per pass, the 9 buffers' pixel slices
        blocks = []
        off = 0
        for FD in PASS_FD:
            for j in range(NB):
                blocks.append(bufs[j][:, off:off + FD])
            off += FD
        bpack = np.ascontiguousarray(np.concatenate(blocks, axis=1))
        in_maps.append({"wt": wtp, "bpack": bpack})
    return in_maps


def assemble_out(res):
    outs = []
    for b in range(B):
        o = res.results[b]["out"]  # [2, 128, PIX] bf16
        outs.append(np.asarray(o, dtype=np.float32).reshape(OUT, H, W))
    return np.stack(outs)


def kernel(x, fc_w):
    x = np.asarray(x, dtype=np.float32)
    fc_w = np.asarray(fc_w, dtype=np.float32)
    nc = _build_module()
    wt = build_wt(fc_w)
    res = run_bass_kernel_spmd(nc, make_in_maps(x, wt), list(range(NCORES)))
    return assemble_out(res)


# revision 17
# speedup vs baseline: 1.0083x; 1.0083x over previous
"""Trainium2 kernel for ChannelQuadLayer.

Per-pixel quadratic channel expansion + 1x1 conv:
    quad = x[:, ii] * x[:, jj]  (all 2080 upper-tri channel pairs)
    y    = concat([x, quad])    -> [B, 2144, H, W]
    out  = einsum('bchw,oc->bohw', y, fc_w)

Strategy (8 NeuronCores, batch-parallel, one sample per core):
  * The 2080 unordered channel pairs are exactly the cyclic diagonals
    d=0..32 of the 64-channel index ring: pairs {i, (i+d)%64}.
  * Host prepares 9 "rotation buffers" B_k = [roll(x,-t_k); roll(x,-u_k)]
    (128 partitions x 4096 pixels, bf16). A single elementwise multiply
    of two such buffers yields TWO complete cyclic diagonals (top half:
    diagonal t_j - t_i, bottom half: u_j - u_i). A difference cover
    (found by search) produces all diagonals 1..32 in 16 multiplies;
    diagonal 0 (squares) comes from one ScalarE Square op.
  * Everything on the input path is bf16: halves HBM traffic (the 9
    rotation buffers are the dominant stream) and doubles VectorE
    multiply throughput; the matmul accumulates in fp32 PSUM so the
    output error stays ~1e-3 (tolerance 2e-2).
  * The 9 buffers are packed pass-major in ONE dram tensor so each
    pixel pass needs only 3 DMA triggers (b0 | b1-4 | b5-8) instead
    of 9 - HWDGE trigger instructions cost ~650ns each on the queue.
    All input DMAs are issued from nc.sync: one HWDGE ring = FIFO, so
    critical early transfers are not bandwidth-shared with prefetches
    (different rings round-robin per packet). Outputs go on the other
    ring (nc.scalar).
  * y-rows: 64 linear + 64 squares + 16*128 pair rows = 2176 = 17*128,
    an exact 17-chunk contraction. fc_w is permuted/padded to this row
    order on the host (duplicate pair rows get zero weight).
  * GEMM: out[256, 4096] = Wt[2176, 256]^T @ y[2176, 4096] on TensorE
    in bf16, accumulating 17 chunks into fp32 PSUM, k-outer so each y
    chunk is consumed right after its producer. Weights DMA in 4
    pieces (chunk 0 | 1-4 | 5-8 | 9-16) interleaved with the pass-0
    buffer pieces in descending urgency, so the first matmul only
    waits on a 64KB transfer.
  * The m=1 PSUM drains run on VectorE so the scalar queue stays free
    for the next pass's chunk-0 activations at pass boundaries.
  * 5 warm-up matmuls on a memset tile bridge the DMA ramp so the PE
    HAM clock-gate (default 1.2 GHz) releases to 2.4 GHz with no idle
    window before the real accumulation starts.
  * Pixel passes [256, 512, 1024, 1024, 1024, 256]: small first passes
    minimize the bytes the PE waits on at startup, the small last pass
    shortens the drain/writeback tail.
"""

import sys

sys.path.insert(0, "/opt/trn_rl_repo")

import ml_dtypes
import numpy as np

import concourse.bass as bass
import concourse.tile as tile
from concourse import bacc, mybir
from concourse.bass_utils import run_bass_kernel_spmd

B, C, H, W = 8, 64, 64, 64
PIX = H * W  # 4096
OUT = 256
NCORES = 8

# rotation difference cover: ops (i,j) give diagonals D(t_j-t_i) (top half)
# and D(u_j-u_i) (bottom half); together exactly {1..32}.
T_ROT = [0, 8, 22, 24, 42, 48, 49, 57, 60]
U_ROT = [0, 59, 16, 38, 55, 22, 30, 54, 35]
OPS = [(1, 3), (2, 3), (1, 4), (2, 4), (3, 4), (4, 5), (1, 6), (2, 6),
       (6, 7), (0, 7), (4, 7), (5, 7), (2, 8), (3, 8), (5, 8), (6, 8)]
NB = len(T_ROT)        # 9 rotation buffers
KCH = 1 + len(OPS)     # 17 contraction chunks of 128 rows
PASS_FD = [256, 512, 1024, 1024, 1024, 256]
assert sum(PASS_FD) == PIX
NPASS = len(PASS_FD)

F32 = mybir.dt.float32
BF16 = mybir.dt.bfloat16
BF16_NP = ml_dtypes.bfloat16

# b-buffer groups: one DMA per group per pass (contiguous in the packed
# dram layout). b0 alone so chunk0 can start as early as possible.
BGRP = [(0, 1), (1, 5), (5, 9)]


def row_pairs():
    """Channel pair (c1, c2) for every global y row, or ('lin', c)."""
    rows = []
    for p in range(128):  # chunk 0
        rows.append(("lin", p) if p < 64 else (p - 64, p - 64))
    for (i, j) in OPS:
        for p in range(128):
            if p < 64:
                c1, c2 = (p + T_ROT[i]) % 64, (p + T_ROT[j]) % 64
            else:
                c1, c2 = (p - 64 + U_ROT[i]) % 64, (p - 64 + U_ROT[j]) % 64
            rows.append((min(c1, c2), max(c1, c2)))
    return rows


def build_wt(fc_w):
    """Permute fc_w [OUT, 2144] into Wt [KCH, 128, OUT] matching y rows."""
    ii, jj = np.triu_indices(C)
    pair2col = {(a, b): C + k for k, (a, b) in enumerate(zip(ii, jj))}
    wt = np.zeros((KCH * 128, OUT), np.float32)
    seen = set()
    for g, r in enumerate(row_pairs()):
        if r[0] == "lin":
            wt[g] = fc_w[:, r[1]]
        elif r not in seen:
            seen.add(r)
            wt[g] = fc_w[:, pair2col[r]]
    assert len(seen) == C * (C + 1) // 2
    return np.ascontiguousarray(wt.reshape(KCH, 128, OUT))


_cached = None


def _build_module():
    global _cached
    if _cached is not None:
        return _cached
    nc = bacc.Bacc("TRN2", target_bir_lowering=False, debug=False,
                   num_devices=NCORES)
    # rotation buffers, pass-major packed: per pass p (pixel range
    # [off, off+FD)), columns [9*off + j*FD, 9*off + (j+1)*FD) hold
    # buffer j's slice.
    bp_d = nc.dram_tensor("bpack", [128, NB * PIX], BF16, kind="ExternalInput")
    # weight matrix, partition-major so DMA rows are contiguous
    wt_d = nc.dram_tensor("wt", [128, KCH * OUT], BF16, kind="ExternalInput")
    out_d = nc.dram_tensor("out", [2, 128, PIX], BF16, kind="ExternalOutput")

    # wt DMA split, in chunks; piece 0 (chunk 0) is tiny so the first
    # real matmul only waits on a 64KB transfer. All INPUT DMAs go on
    # the nc.sync HWDGE ring: one ring = FIFO, so earlier (critical)
    # transfers are not bandwidth-shared with later prefetches.
    WPIECE = [(0, 1), (1, 5), (5, 9), (9, KCH)]

    with tile.TileContext(nc) as tc:
        with tc.tile_pool(name="wt", bufs=1) as wt_pool, \
             tc.tile_pool(name="warm", bufs=1) as warm_pool, \
             tc.tile_pool(name="bsrc", bufs=2) as b_pool, \
             tc.tile_pool(name="y", bufs=8) as y_pool, \
             tc.tile_pool(name="ostage", bufs=4) as o_pool, \
             tc.tile_pool(name="psum", bufs=8, space="PSUM") as ps_pool:

            wt_t = wt_pool.tile([128, KCH * OUT], BF16, name="wtt")

            # PE warm-up: ~5 cold matmuls bridge the DMA ramp so the HAM
            # clock gate sees activity early. Results are never read.
            wz = warm_pool.tile([128, 512], BF16, name="warmz")
            nc.gpsimd.memset(wz[:, :], 0.0)
            ps_w = ps_pool.tile([128, 512], F32, tag="ps", name="ps_warm")
            for _ in range(5):
                nc.tensor.matmul(ps_w[:, :], wz[:, 0:128], wz[:, :],
                                 start=True, stop=True)

            def wt_dma(piece):
                k0, k1 = WPIECE[piece]
                nc.sync.dma_start(wt_t[:, k0 * OUT:k1 * OUT],
                                  wt_d.ap()[:, k0 * OUT:k1 * OUT])

            PASS_OFF = [sum(PASS_FD[:p]) for p in range(NPASS)]

            def issue_pass(ps):
                """Allocate + DMA the 3 b-buffer groups of pass ps.

                One tile per buffer-group; 3 DMA triggers per pass,
                FIFO-ordered so the critical pieces land first."""
                FD = PASS_FD[ps]
                base = NB * PASS_OFF[ps]
                gt = []
                for gi, (j0, j1) in enumerate(BGRP):
                    t = b_pool.tile([128, (j1 - j0) * 1024], BF16,
                                    tag=f"bg{gi}", name=f"bg{gi}_{ps}")
                    nc.sync.dma_start(
                        t[:, :(j1 - j0) * FD],
                        bp_d.ap()[:, base + j0 * FD:base + j1 * FD])
                    gt.append(t)
                    if ps == 0 and gi == 1:
                        wt_dma(1)
                        wt_dma(2)
                    if ps == 0 and gi == 2:
                        wt_dma(3)
                return gt

            def produce_y0(ps, gt):
                """Chunk-0 y (linear rows + squares) from the b0 tile.

                Called one pass AHEAD (software pipeline) so the chunk-0
                activations never sit behind the previous pass's drains
                on the scalar queue at a pass boundary. In pass 0 the
                copy goes on VectorE so the two halves build
                concurrently (startup critical path)."""
                FD = PASS_FD[ps]
                yk = y_pool.tile([128, 1024], BF16, tag="y", name=f"y{ps}_0")
                b0 = gt[0]
                if ps == 0:
                    nc.vector.tensor_copy(yk[0:64, :FD], b0[0:64, :FD])
                else:
                    nc.scalar.activation(
                        yk[0:64, :FD], b0[0:64, :FD],
                        mybir.ActivationFunctionType.Identity)
                nc.scalar.activation(
                    yk[64:128, :FD], b0[64:128, :FD],
                    mybir.ActivationFunctionType.Square)
                return yk

            wt_dma(0)
            gt_next = issue_pass(0)
            y0_next = produce_y0(0, gt_next)

            off = 0
            for ps, FD in enumerate(PASS_FD):
                NT = max(1, FD // 512)
                NW = min(512, FD)  # matmul free width
                gt = gt_next
                y0 = y0_next

                def bview(j, gt=gt, FD=FD):
                    for gi, (j0, j1) in enumerate(BGRP):
                        if j0 <= j < j1:
                            return gt[gi][:, (j - j0) * FD:(j - j0 + 1) * FD]
                    raise AssertionError

                psum = [ps_pool.tile([128, 512], F32, tag="ps",
                                     name=f"ps{ps}_{g}")
                        for g in range(2 * NT)]

                for k in range(KCH):
                    if k == 0:
                        yk = y0
                    else:
                        yk = y_pool.tile([128, 1024], BF16, tag="y",
                                         name=f"y{ps}_{k}")
                        i, j = OPS[k - 1]
                        nc.vector.tensor_mul(yk[:, :FD], bview(i), bview(j))
                    for m in range(2):
                        lhsT = wt_t[:, k * OUT + m * 128:k * OUT + (m + 1) * 128]
                        for n in range(NT):
                            nc.tensor.matmul(
                                psum[m * NT + n][:, :NW],
                                lhsT,
                                yk[:, n * NW:(n + 1) * NW],
                                start=(k == 0), stop=(k == KCH - 1))

                # next pass's DMAs + chunk-0 production BEFORE this
                # pass's drains (software pipeline across passes)
                if ps + 1 < NPASS:
                    gt_next = issue_pass(ps + 1)
                    y0_next = produce_y0(ps + 1, gt_next)

                last = ps == NPASS - 1
                for m in range(2):
                    ot = o_pool.tile([128, 1024], BF16, tag="ostage",
                                     name=f"o{ps}_{m}")
                    for n in range(NT):
                        src = psum[m * NT + n][:, :NW]
                        dst = ot[:, n * NW:(n + 1) * NW]
                        # m1 drains on VectorE so the scalar queue is free
                        # for the next pass's chunk-0 activations at the
                        # pass boundary; m0 stays on ScalarE.
                        if m == 1:
                            nc.vector.tensor_copy(dst, src)
                        else:
                            nc.scalar.activation(
                                dst, src, mybir.ActivationFunctionType.Identity)
                    eng = nc.sync if (last and m == 1) else nc.scalar
                    eng.dma_start(out_d.ap()[m, :, off:off + FD], ot[:, :FD])
                off += FD
    nc.compile()
    _cached = nc
    return nc


def make_in_maps(x, wt):
    # [KCH, 128, OUT] -> [128, KCH*OUT], bf16
    wtp = np.ascontiguousarray(
        wt.transpose(1, 0, 2).reshape(128, KCH * OUT).astype(BF16_NP))
    in_maps = []
    for b in range(B):
        xc = np.asarray(x[b], np.float32).reshape(C, PIX).astype(BF16_NP)
        bufs = [np.concatenate(
            [np.roll(xc, -t, axis=0), np.roll(xc, -u, axis=0)])
            for t, u in zip(T_ROT, U_ROT)]
        # pass-major packing: per pass, the 9 buffers' pixel slices
        blocks = []
        off = 0
        for FD in PASS_FD:
            for j in range(NB):
                blocks.append(bufs[j][:, off:off + FD])
            off += FD
        bpack = np.ascontiguousarray(np.concatenate(blocks, axis=1))
        in_maps.append({"wt": wtp, "bpack": bpack})
    return in_maps


def assemble_out(res):
    outs = []
    for b in range(B):
        o = res.results[b]["out"]  # [2, 128, PIX] bf16
        outs.append(np.asarray(o, dtype=np.float32).reshape(OUT, H, W))
    return np.stack(outs)


def kernel(x, fc_w):
    x = np.asarray(x, dtype=np.float32)
    fc_w = np.asarray(fc_w, dtype=np.float32)
    nc = _build_module()
    wt = build_wt(fc_w)
    res = run_bass_kernel_spmd(nc, make_in_maps(x, wt), list(range(NCORES)))
    return assemble_out(res)


# revision 18
# speedup vs baseline: 1.1640x; 1.1544x over previous
"""Trainium2 kernel for ChannelQuadLayer.

Per-pixel quadratic channel expansion + 1x1 conv:
    quad = x[:, ii] * x[:, jj]  (all 2080 upper-tri channel pairs)
    y    = concat([x, quad])    -> [B, 2144, H, W]
    out  = einsum('bchw,oc->bohw', y, fc_w)

Strategy (8 NeuronCores, batch-parallel, one sample per core):
  * The 2080 unordered channel pairs are exactly the cyclic diagonals
    d=0..32 of the 64-channel index ring: pairs {i, (i+d)%64}.
  * Host prepares 9 "rotation buffers" B_k = [roll(x,-t_k); roll(x,-u_k)]
    (128 partitions x 4096 pixels, bf16). A single elementwise multiply
    of two such buffers yields TWO complete cyclic diagonals (top half:
    diagonal t_j - t_i, bottom half: u_j - u_i). A difference cover
    (found by search) produces all diagonals 1..32 in 16 multiplies;
    diagonal 0 (squares) comes from one ScalarE Square op.
  * Everything on the input path is bf16: halves HBM traffic (the 9
    rotation buffers are the dominant stream) and doubles VectorE
    multiply throughput; the matmul accumulates in fp32 PSUM so the
    output error stays ~1e-3 (tolerance 2e-2).
  * The 9 buffers are packed pass-major in ONE dram tensor so each
    pixel pass needs only 3 DMA triggers (b0 | b1-4 | b5-8) instead
    of 9 - HWDGE trigger instructions cost ~650ns each on the queue.
    All input DMAs are issued from nc.sync: one HWDGE ring = FIFO, so
    critical early transfers are not bandwidth-shared with prefetches
    (different rings round-robin per packet). Outputs go on the other
    ring (nc.scalar).
  * y-rows: 64 linear + 64 squares + 16*128 pair rows = 2176 = 17*128,
    an exact 17-chunk contraction. fc_w is permuted/padded to this row
    order on the host (duplicate pair rows get zero weight).
  * GEMM: out[256, 4096] = Wt[2176, 256]^T @ y[2176, 4096] on TensorE
    in bf16, accumulating 17 chunks into fp32 PSUM, k-outer so each y
    chunk is consumed right after its producer. Weights DMA in 4
    pieces (chunk 0 | 1-4 | 5-8 | 9-16) interleaved with the pass-0
    buffer pieces in descending urgency, so the first matmul only
    waits on a 64KB transfer.
  * The m=1 PSUM drains run on VectorE so the scalar queue stays free
    for the next pass's chunk-0 activations at pass boundaries.
  * 5 warm-up matmuls on a memset tile bridge the DMA ramp so the PE
    HAM clock-gate (default 1.2 GHz) releases to 2.4 GHz with no idle
    window before the real accumulation starts.
  * Pixel passes [256, 512, 1024, 1024, 1024, 256]: small first passes
    minimize the bytes the PE waits on at startup, the small last pass
    shortens the drain/writeback tail.
"""

import sys

sys.path.insert(0, "/opt/trn_rl_repo")

import ml_dtypes
import numpy as np

import concourse.bass as bass
import concourse.tile as tile
from concourse import bacc, mybir
from concourse.bass_utils import run_bass_kernel_spmd

B, C, H, W = 8, 64, 64, 64
PIX = H * W  # 4096
OUT = 256
NCORES = 8

# rotation difference cover: ops (i,j) give diagonals D(t_j-t_i) (top half)
# and D(u_j-u_i) (bottom half); together exactly {1..32}.
T_ROT = [0, 8, 22, 24, 42, 48, 49, 57, 60]
U_ROT = [0, 59, 16, 38, 55, 22, 30, 54, 35]
OPS = [(1, 3), (2, 3), (1, 4), (2, 4), (3, 4), (4, 5), (1, 6), (2, 6),
       (6, 7), (0, 7), (4, 7), (5, 7), (2, 8), (3, 8), (5, 8), (6, 8)]
NB = len(T_ROT)        # 9 rotation buffers
KCH = 1 + len(OPS)     # 17 contraction chunks of 128 rows
PASS_FD = [256, 512, 1024, 1024, 1024, 256]
assert sum(PASS_FD) == PIX
NPASS = len(PASS_FD)

F32 = mybir.dt.float32
BF16 = mybir.dt.bfloat16
BF16_NP = ml_dtypes.bfloat16

# b-buffer groups: one DMA per group per pass (contiguous in the packed
# dram layout). b0 alone so chunk0 can start as early as possible.
BGRP = [(0, 1), (1, 5), (5, 9)]


def row_pairs():
    """Channel pair (c1, c2) for every global y row, or ('lin', c)."""
    rows = []
    for p in range(128):  # chunk 0
        rows.append(("lin", p) if p < 64 else (p - 64, p - 64))
    for (i, j) in OPS:
        for p in range(128):
            if p < 64:
                c1, c2 = (p + T_ROT[i]) % 64, (p + T_ROT[j]) % 64
            else:
                c1, c2 = (p - 64 + U_ROT[i]) % 64, (p - 64 + U_ROT[j]) % 64
            rows.append((min(c1, c2), max(c1, c2)))
    return rows


def build_wt(fc_w):
    """Permute fc_w [OUT, 2144] into Wt [KCH, 128, OUT] matching y rows."""
    ii, jj = np.triu_indices(C)
    pair2col = {(a, b): C + k for k, (a, b) in enumerate(zip(ii, jj))}
    wt = np.zeros((KCH * 128, OUT), np.float32)
    seen = set()
    for g, r in enumerate(row_pairs()):
        if r[0] == "lin":
            wt[g] = fc_w[:, r[1]]
        elif r not in seen:
            seen.add(r)
            wt[g] = fc_w[:, pair2col[r]]
    assert len(seen) == C * (C + 1) // 2
    return np.ascontiguousarray(wt.reshape(KCH, 128, OUT))


_cached = None


def _build_module():
    global _cached
    if _cached is not None:
        return _cached
    nc = bacc.Bacc("TRN2", target_bir_lowering=False, debug=False,
                   num_devices=NCORES)
    # rotation buffers, pass-major packed: per pass p (pixel range
    # [off, off+FD)), columns [9*off + j*FD, 9*off + (j+1)*FD) hold
    # buffer j's slice.
    bp_d = nc.dram_tensor("bpack", [128, NB * PIX], BF16, kind="ExternalInput")
    # weight matrix, partition-major so DMA rows are contiguous
    wt_d = nc.dram_tensor("wt", [128, KCH * OUT], BF16, kind="ExternalInput")
    out_d = nc.dram_tensor("out", [2, 128, PIX], BF16, kind="ExternalOutput")

    # wt DMA split, in chunks; piece 0 (chunk 0) is tiny so the first
    # real matmul only waits on a 64KB transfer. All INPUT DMAs go on
    # the nc.sync HWDGE ring: one ring = FIFO, so earlier (critical)
    # transfers are not bandwidth-shared with later prefetches.
    WPIECE = [(0, 1), (1, 5), (5, 9), (9, KCH)]

    with tile.TileContext(nc) as tc:
        with tc.tile_pool(name="wt", bufs=1) as wt_pool, \
             tc.tile_pool(name="warm", bufs=1) as warm_pool, \
             tc.tile_pool(name="bsrc", bufs=2) as b_pool, \
             tc.tile_pool(name="y", bufs=8) as y_pool, \
             tc.tile_pool(name="ostage", bufs=4) as o_pool, \
             tc.tile_pool(name="psum", bufs=8, space="PSUM") as ps_pool:

            wt_t = wt_pool.tile([128, KCH * OUT], BF16, name="wtt")

            # PE warm-up: ~5 cold matmuls bridge the DMA ramp so the HAM
            # clock gate sees activity early. Results are never read.
            wz = warm_pool.tile([128, 512], BF16, name="warmz")
            nc.gpsimd.memset(wz[:, :], 0.0)
            ps_w = ps_pool.tile([128, 512], F32, tag="ps", name="ps_warm")
            for _ in range(5):
                nc.tensor.matmul(ps_w[:, :], wz[:, 0:128], wz[:, :],
                                 start=True, stop=True)

            def wt_dma(piece):
                k0, k1 = WPIECE[piece]
                nc.sync.dma_start(wt_t[:, k0 * OUT:k1 * OUT],
                                  wt_d.ap()[:, k0 * OUT:k1 * OUT])

            PASS_OFF = [sum(PASS_FD[:p]) for p in range(NPASS)]

            def issue_pass(ps):
                """Allocate + DMA the 3 b-buffer groups of pass ps.

                One tile per buffer-group; 3 DMA triggers per pass,
                FIFO-ordered so the critical pieces land first."""
                FD = PASS_FD[ps]
                base = NB * PASS_OFF[ps]
                gt = []
                for gi, (j0, j1) in enumerate(BGRP):
                    t = b_pool.tile([128, (j1 - j0) * 1024], BF16,
                                    tag=f"bg{gi}", name=f"bg{gi}_{ps}")
                    nc.sync.dma_start(
                        t[:, :(j1 - j0) * FD],
                        bp_d.ap()[:, base + j0 * FD:base + j1 * FD])
                    gt.append(t)
                    if ps == 0 and gi == 1:
                        wt_dma(1)
                        wt_dma(2)
                    if ps == 0 and gi == 2:
                        wt_dma(3)
                return gt

            def produce_y0(ps, gt):
                """Chunk-0 y (linear rows + squares) from the b0 tile.

                Called one pass AHEAD (software pipeline) so the chunk-0
                activations never sit behind the previous pass's drains
                on the scalar queue at a pass boundary. In pass 0 the
                copy goes on VectorE so the two halves build
                concurrently (startup critical path)."""
                FD = PASS_FD[ps]
                yk = y_pool.tile([128, 1024], BF16, tag="y", name=f"y{ps}_0")
                b0 = gt[0]
                if ps == 0:
                    nc.vector.tensor_copy(yk[0:64, :FD], b0[0:64, :FD])
                else:
                    nc.scalar.activation(
                        yk[0:64, :FD], b0[0:64, :FD],
                        mybir.ActivationFunctionType.Identity)
                nc.scalar.activation(
                    yk[64:128, :FD], b0[64:128, :FD],
                    mybir.ActivationFunctionType.Square)
                return yk

            wt_dma(0)
            gt_next = issue_pass(0)
            y0_next = produce_y0(0, gt_next)

            off = 0
            for ps, FD in enumerate(PASS_FD):
                NT = max(1, FD // 512)
                NW = min(512, FD)  # matmul free width
                gt = gt_next
                y0 = y0_next

                def bview(j, gt=gt, FD=FD):
                    for gi, (j0, j1) in enumerate(BGRP):
                        if j0 <= j < j1:
                            return gt[gi][:, (j - j0) * FD:(j - j0 + 1) * FD]
                    raise AssertionError

                psum = [ps_pool.tile([128, 512], F32, tag="ps",
                                     name=f"ps{ps}_{g}")
                        for g in range(2 * NT)]

                for k in range(KCH):
                    if k == 0:
                        yk = y0
                    else:
                        yk = y_pool.tile([128, 1024], BF16, tag="y",
                                         name=f"y{ps}_{k}")
                        i, j = OPS[k - 1]
                        # In the wide passes VectorE production (~1030ns
                        # per chunk) exactly ties PE consumption; hand 2
                        # of the 16 multiplies to the otherwise-idle
                        # GpSimd so VectorE keeps a cushion.
                        eng = nc.gpsimd if (FD == 1024 and k in (6, 12)) \
                            else nc.vector
                        eng.tensor_mul(yk[:, :FD], bview(i), bview(j))
                    for m in range(2):
                        lhsT = wt_t[:, k * OUT + m * 128:k * OUT + (m + 1) * 128]
                        for n in range(NT):
                            nc.tensor.matmul(
                                psum[m * NT + n][:, :NW],
                                lhsT,
                                yk[:, n * NW:(n + 1) * NW],
                                start=(k == 0), stop=(k == KCH - 1))

                # next pass's DMAs + chunk-0 production BEFORE this
                # pass's drains (software pipeline across passes)
                if ps + 1 < NPASS:
                    gt_next = issue_pass(ps + 1)
                    y0_next = produce_y0(ps + 1, gt_next)

                last = ps == NPASS - 1
                for m in range(2):
                    ot = o_pool.tile([128, 1024], BF16, tag="ostage",
                                     name=f"o{ps}_{m}")
                    for n in range(NT):
                        src = psum[m * NT + n][:, :NW]
                        dst = ot[:, n * NW:(n + 1) * NW]
                        # m1 drains on VectorE so the scalar queue is free
                        # for the next pass's chunk-0 activations at the
                        # pass boundary; m0 stays on ScalarE.
                        if m == 1:
                            nc.vector.tensor_copy(dst, src)
                        else:
                            nc.scalar.activation(
                                dst, src, mybir.ActivationFunctionType.Identity)
                    eng = nc.sync if (last and m == 1) else nc.scalar
                    eng.dma_start(out_d.ap()[m, :, off:off + FD], ot[:, :FD])
                off += FD
    nc.compile()
    _cached = nc
    return nc


def make_in_maps(x, wt):
    # [KCH, 128, OUT] -> [128, KCH*OUT], bf16
    wtp = np.ascontiguousarray(
        wt.transpose(1, 0, 2).reshape(128, KCH * OUT).astype(BF16_NP))
    in_maps = []
    for b in range(B):
        xc = np.asarray(x[b], np.float32).reshape(C, PIX).astype(BF16_NP)
        bufs = [np.concatenate(
            [np.roll(xc, -t, axis=0), np.roll(xc, -u, axis=0)])
            for t, u in zip(T_ROT, U_ROT)]
        # pass-major packing: per pass, the 9 buffers' pixel slices
        blocks = []
        off = 0
        for FD in PASS_FD:
            for j in range(NB):
                blocks.append(bufs[j][:, off:off + FD])
            off += FD
        bpack = np.ascontiguousarray(np.concatenate(blocks, axis=1))
        in_maps.append({"wt": wtp, "bpack": bpack})
    return in_maps


def assemble_out(res):
    outs = []
    for b in range(B):
        o = res.results[b]["out"]  # [2, 128, PIX] bf16
        outs.append(np.asarray(o, dtype=np.float32).reshape(OUT, H, W))
    return np.stack(outs)


def kernel(x, fc_w):
    x = np.asarray(x, dtype=np.float32)
    fc_w = np.asarray(fc_w, dtype=np.float32)
    nc = _build_module()
    wt = build_wt(fc_w)
    res = run_bass_kernel_spmd(nc, make_in_maps(x, wt), list(range(NCORES)))
    return assemble_out(res)


# revision 20
# speedup vs baseline: 1.1752x; 1.0096x over previous
"""Trainium2 kernel for ChannelQuadLayer.

Per-pixel quadratic channel expansion + 1x1 conv:
    quad = x[:, ii] * x[:, jj]  (all 2080 upper-tri channel pairs)
    y    = concat([x, quad])    -> [B, 2144, H, W]
    out  = einsum('bchw,oc->bohw', y, fc_w)

Strategy (8 NeuronCores, batch-parallel, one sample per core):
  * The 2080 unordered channel pairs are exactly the cyclic diagonals
    d=0..32 of the 64-channel index ring: pairs {i, (i+d)%64}.
  * Host prepares 9 "rotation buffers" B_k = [roll(x,-t_k); roll(x,-u_k)]
    (128 partitions x 4096 pixels, bf16). A single elementwise multiply
    of two such buffers yields TWO complete cyclic diagonals (top half:
    diagonal t_j - t_i, bottom half: u_j - u_i). A difference cover
    (found by search) produces all diagonals 1..32 in 16 multiplies;
    diagonal 0 (squares) comes from one ScalarE Square op.
  * Everything on the input path is bf16: halves HBM traffic (the 9
    rotation buffers are the dominant stream) and doubles VectorE
    multiply throughput; the matmul accumulates in fp32 PSUM so the
    output error stays ~1e-3 (tolerance 2e-2).
  * The 9 buffers are packed pass-major in ONE dram tensor so each
    pixel pass needs only 3 DMA triggers (b0 | b1-4 | b5-8) instead
    of 9 - HWDGE trigger instructions cost ~650ns each on the queue.
    All input DMAs are issued from nc.sync: one HWDGE ring = FIFO, so
    critical early transfers are not bandwidth-shared with prefetches
    (different rings round-robin per packet). Outputs go on the other
    ring (nc.scalar).
  * y-rows: 64 linear + 64 squares + 16*128 pair rows = 2176 = 17*128,
    an exact 17-chunk contraction. fc_w is permuted/padded to this row
    order on the host (duplicate pair rows get zero weight).
  * GEMM: out[256, 4096] = Wt[2176, 256]^T @ y[2176, 4096] on TensorE
    in bf16, accumulating 17 chunks into fp32 PSUM, k-outer so each y
    chunk is consumed right after its producer. Weights DMA in 4
    pieces (chunk 0 | 1-4 | 5-8 | 9-16) interleaved with the pass-0
    buffer pieces in descending urgency, so the first matmul only
    waits on a 64KB transfer.
  * The m=1 PSUM drains run on VectorE so the scalar queue stays free
    for the next pass's chunk-0 activations at pass boundaries.
  * 5 warm-up matmuls on a memset tile bridge the DMA ramp so the PE
    HAM clock-gate (default 1.2 GHz) releases to 2.4 GHz with no idle
    window before the real accumulation starts.
  * Pixel passes [256, 512, 1024, 1024, 1024, 256]: small first passes
    minimize the bytes the PE waits on at startup, the small last pass
    shortens the drain/writeback tail.
"""

import sys

sys.path.insert(0, "/opt/trn_rl_repo")

import ml_dtypes
import numpy as np

import concourse.bass as bass
import concourse.tile as tile
from concourse import bacc, mybir
from concourse.bass_utils import run_bass_kernel_spmd

B, C, H, W = 8, 64, 64, 64
PIX = H * W  # 4096
OUT = 256
NCORES = 8

# rotation difference cover: ops (i,j) give diagonals D(t_j-t_i) (top half)
# and D(u_j-u_i) (bottom half); together exactly {1..32}.
T_ROT = [0, 8, 22, 24, 42, 48, 49, 57, 60]
U_ROT = [0, 59, 16, 38, 55, 22, 30, 54, 35]
OPS = [(1, 3), (2, 3), (1, 4), (2, 4), (3, 4), (4, 5), (1, 6), (2, 6),
       (6, 7), (0, 7), (4, 7), (5, 7), (2, 8), (3, 8), (5, 8), (6, 8)]
NB = len(T_ROT)        # 9 rotation buffers
KCH = 1 + len(OPS)     # 17 contraction chunks of 128 rows
PASS_FD = [512, 1024, 1024, 1024, 256, 256]
assert sum(PASS_FD) == PIX
NPASS = len(PASS_FD)

F32 = mybir.dt.float32
BF16 = mybir.dt.bfloat16
BF16_NP = ml_dtypes.bfloat16

# b-buffer groups: one DMA per group per pass (contiguous in the packed
# dram layout). b0 alone so chunk0 can start as early as possible.
BGRP = [(0, 1), (1, 5), (5, 9)]


def row_pairs():
    """Channel pair (c1, c2) for every global y row, or ('lin', c)."""
    rows = []
    for p in range(128):  # chunk 0
        rows.append(("lin", p) if p < 64 else (p - 64, p - 64))
    for (i, j) in OPS:
        for p in range(128):
            if p < 64:
                c1, c2 = (p + T_ROT[i]) % 64, (p + T_ROT[j]) % 64
            else:
                c1, c2 = (p - 64 + U_ROT[i]) % 64, (p - 64 + U_ROT[j]) % 64
            rows.append((min(c1, c2), max(c1, c2)))
    return rows


def build_wt(fc_w):
    """Permute fc_w [OUT, 2144] into Wt [KCH, 128, OUT] matching y rows."""
    ii, jj = np.triu_indices(C)
    pair2col = {(a, b): C + k for k, (a, b) in enumerate(zip(ii, jj))}
    wt = np.zeros((KCH * 128, OUT), np.float32)
    seen = set()
    for g, r in enumerate(row_pairs()):
        if r[0] == "lin":
            wt[g] = fc_w[:, r[1]]
        elif r not in seen:
            seen.add(r)
            wt[g] = fc_w[:, pair2col[r]]
    assert len(seen) == C * (C + 1) // 2
    return np.ascontiguousarray(wt.reshape(KCH, 128, OUT))


_cached = None


def _build_module():
    global _cached
    if _cached is not None:
        return _cached
    nc = bacc.Bacc("TRN2", target_bir_lowering=False, debug=False,
                   num_devices=NCORES)
    # rotation buffers, pass-major packed: per pass p (pixel range
    # [off, off+FD)), columns [9*off + j*FD, 9*off + (j+1)*FD) hold
    # buffer j's slice.
    bp_d = nc.dram_tensor("bpack", [128, NB * PIX], BF16, kind="ExternalInput")
    # weight matrix, partition-major so DMA rows are contiguous
    wt_d = nc.dram_tensor("wt", [128, KCH * OUT], BF16, kind="ExternalInput")
    out_d = nc.dram_tensor("out", [2, 128, PIX], BF16, kind="ExternalOutput")

    # wt DMA split, in chunks; piece 0 (chunk 0) is tiny so the first
    # real matmul only waits on a 64KB transfer. All INPUT DMAs go on
    # the nc.sync HWDGE ring: one ring = FIFO, so earlier (critical)
    # transfers are not bandwidth-shared with later prefetches.
    WPIECE = [(0, 1), (1, 5), (5, 9), (9, KCH)]

    with tile.TileContext(nc) as tc:
        with tc.tile_pool(name="wt", bufs=1) as wt_pool, \
             tc.tile_pool(name="warm", bufs=1) as warm_pool, \
             tc.tile_pool(name="bsrc", bufs=2) as b_pool, \
             tc.tile_pool(name="y", bufs=8) as y_pool, \
             tc.tile_pool(name="ostage", bufs=4) as o_pool, \
             tc.tile_pool(name="psum", bufs=8, space="PSUM") as ps_pool:

            wt_t = wt_pool.tile([128, KCH * OUT], BF16, name="wtt")

            # PE warm-up: ~5 cold matmuls bridge the DMA ramp so the HAM
            # clock gate sees activity early. Results are never read.
            wz = warm_pool.tile([128, 512], BF16, name="warmz")
            nc.gpsimd.memset(wz[:, :], 0.0)
            ps_w = ps_pool.tile([128, 512], F32, tag="ps", name="ps_warm")
            for _ in range(5):
                nc.tensor.matmul(ps_w[:, :], wz[:, 0:128], wz[:, :],
                                 start=True, stop=True)

            def wt_dma(piece):
                k0, k1 = WPIECE[piece]
                nc.sync.dma_start(wt_t[:, k0 * OUT:k1 * OUT],
                                  wt_d.ap()[:, k0 * OUT:k1 * OUT])

            PASS_OFF = [sum(PASS_FD[:p]) for p in range(NPASS)]

            def issue_pass(ps):
                """Allocate + DMA the 3 b-buffer groups of pass ps.

                One tile per buffer-group; 3 DMA triggers per pass,
                FIFO-ordered so the critical pieces land first."""
                FD = PASS_FD[ps]
                base = NB * PASS_OFF[ps]
                gt = []
                for gi, (j0, j1) in enumerate(BGRP):
                    t = b_pool.tile([128, (j1 - j0) * 1024], BF16,
                                    tag=f"bg{gi}", name=f"bg{gi}_{ps}")
                    nc.sync.dma_start(
                        t[:, :(j1 - j0) * FD],
                        bp_d.ap()[:, base + j0 * FD:base + j1 * FD])
                    gt.append(t)
                    if ps == 0 and gi == 1:
                        wt_dma(1)
                        wt_dma(2)
                    if ps == 0 and gi == 2:
                        wt_dma(3)
                return gt

            def produce_y0(ps, gt):
                """Chunk-0 y (linear rows + squares) from the b0 tile.

                Called one pass AHEAD (software pipeline) so the chunk-0
                activations never sit behind the previous pass's drains
                on the scalar queue at a pass boundary. In pass 0 the
                copy goes on VectorE so the two halves build
                concurrently (startup critical path)."""
                FD = PASS_FD[ps]
                yk = y_pool.tile([128, 1024], BF16, tag="y", name=f"y{ps}_0")
                b0 = gt[0]
                if ps == 0:
                    nc.vector.tensor_copy(yk[0:64, :FD], b0[0:64, :FD])
                else:
                    nc.scalar.activation(
                        yk[0:64, :FD], b0[0:64, :FD],
                        mybir.ActivationFunctionType.Identity)
                nc.scalar.activation(
                    yk[64:128, :FD], b0[64:128, :FD],
                    mybir.ActivationFunctionType.Square)
                return yk

            wt_dma(0)
            gt_next = issue_pass(0)
            y0_next = produce_y0(0, gt_next)

            off = 0
            for ps, FD in enumerate(PASS_FD):
                NT = max(1, FD // 512)
                NW = min(512, FD)  # matmul free width
                gt = gt_next
                y0 = y0_next

                def bview(j, gt=gt, FD=FD):
                    for gi, (j0, j1) in enumerate(BGRP):
                        if j0 <= j < j1:
                            return gt[gi][:, (j - j0) * FD:(j - j0 + 1) * FD]
                    raise AssertionError

                psum = [ps_pool.tile([128, 512], F32, tag="ps",
                                     name=f"ps{ps}_{g}")
                        for g in range(2 * NT)]

                for k in range(KCH):
                    if k == 0:
                        yk = y0
                    else:
                        yk = y_pool.tile([128, 1024], BF16, tag="y",
                                         name=f"y{ps}_{k}")
                        i, j = OPS[k - 1]
                        nc.vector.tensor_mul(yk[:, :FD], bview(i), bview(j))
                    for m in range(2):
                        lhsT = wt_t[:, k * OUT + m * 128:k * OUT + (m + 1) * 128]
                        for n in range(NT):
                            nc.tensor.matmul(
                                psum[m * NT + n][:, :NW],
                                lhsT,
                                yk[:, n * NW:(n + 1) * NW],
                                start=(k == 0), stop=(k == KCH - 1))

                # next pass's DMAs + chunk-0 production BEFORE this
                # pass's drains (software pipeline across passes)
                if ps + 1 < NPASS:
                    gt_next = issue_pass(ps + 1)
                    y0_next = produce_y0(ps + 1, gt_next)

                last = ps == NPASS - 1
                for m in range(2):
                    ot = o_pool.tile([128, 1024], BF16, tag="ostage",
                                     name=f"o{ps}_{m}")
                    for n in range(NT):
                        src = psum[m * NT + n][:, :NW]
                        dst = ot[:, n * NW:(n + 1) * NW]
                        # m1 drains on VectorE so the scalar queue is free
                        # for the next pass's chunk-0 activations at the
                        # pass boundary; m0 stays on ScalarE.
                        if m == 1:
                            nc.vector.tensor_copy(dst, src)
                        else:
                            nc.scalar.activation(
                                dst, src, mybir.ActivationFunctionType.Identity)
                    eng = nc.sync if (last and m == 1) else nc.scalar
                    eng.dma_start(out_d.ap()[m, :, off:off + FD], ot[:, :FD])
                off += FD
    nc.compile()
    _cached = nc
    return nc


def make_in_maps(x, wt):
    # [KCH, 128, OUT] -> [128, KCH*OUT], bf16
    wtp = np.ascontiguousarray(
        wt.transpose(1, 0, 2).reshape(128, KCH * OUT).astype(BF16_NP))
    in_maps = []
    for b in range(B):
        xc = np.asarray(x[b], np.float32).reshape(C, PIX).astype(BF16_NP)
        bufs = [np.concatenate(
            [np.roll(xc, -t, axis=0), np.roll(xc, -u, axis=0)])
            for t, u in zip(T_ROT, U_ROT)]
        # pass-major packing: per pass, the 9 buffers' pixel slices
        blocks = []
        off = 0
        for FD in PASS_FD:
            for j in range(NB):
                blocks.append(bufs[j][:, off:off + FD])
            off += FD
        bpack = np.ascontiguousarray(np.concatenate(blocks, axis=1))
        in_maps.append({"wt": wtp, "bpack": bpack})
    return in_maps


def assemble_out(res):
    outs = []
    for b in range(B):
        o = res.results[b]["out"]  # [2, 128, PIX] bf16
        outs.append(np.asarray(o, dtype=np.float32).reshape(OUT, H, W))
    return np.stack(outs)


def kernel(x, fc_w):
    x = np.asarray(x, dtype=np.float32)
    fc_w = np.asarray(fc_w, dtype=np.float32)
    nc = _build_module()
    wt = build_wt(fc_w)
    res = run_bass_kernel_spmd(nc, make_in_maps(x, wt), list(range(NCORES)))
    return assemble_out(res)
